# revision 59
# baseline (speedup 1.0000x reference)
"""Discriminative-loss kernel for Trainium2 (Bass/Tile), 8-core data-parallel.

One core per batch sample.  All label-derived tensors and all x re-layouts
(cast / transpose / label-sort) are prepared on the host; the device streams
each tensor exactly once from HBM with large contiguous DMAs.

Device program per core (N = 262144 points, d = 16, K = 8):
  phase B  cluster sums: 256 "superblock" matmuls.  Stationary = x slab
           [128 pts, (b,dd)=128] fp8, moving = onehot [128 pts, (b,k)=64]
           fp8, accumulated into one PSUM region.  Diagonal blocks b==b'
           hold per-slot cluster sums; cross terms are discarded.  The 8
           diagonal [16,8] blocks are summed by 8 tiny accumulating
           matmuls against identity slices -> S[dd,k]; two more tiny
           matmuls replicate S into a per-partition bias vector
           cbias[16j+dd, k] = -S[k,dd]/m_k  (counts m come from host).
  pass 2   x is shipped SORTED BY LABEL per chunk j (segments padded to
           PSEG columns, pad = 0).  Within segment k the own-center is
           constant, so dist^2 needs no gather/subtract:
             sq = ACT Square(x + cbias[:,k])          (one op per segment)
             dist^2 = col-tiled reduce matmuls over dd -> PSUM bank
             e = ACT sqrt(bank)  ->  relu(e-1)^2 + per-label masked
             accumulation (labels in matching layout) -> V partials,
           all per-bank, overlapped with the squares of later chunks.
           Pad points give dist ~ 0 < delta_var so the relu zeroes them.
  host     centers / dist / reg terms + final mean from S, V, counts
           (O(K^2 d) flops on reduced stats only).
"""

import contextlib
import ctypes
import sys
import types

import numpy as np

# ---------------------------------------------------------------------------
# problem constants (hardcoded per contract)
B, D, HH, WW, K = 8, 16, 512, 512, 8
N = HH * WW            # 262144 points per sample
J = 8                  # chunk rows: x row = 16*j + dd
NCORES = 8
DELTA_VAR = 1.0
DELTA_DIST = 2.0
NCH = 4                # DMA chunks per big tensor

_ML = None


def _mld():
    global _ML
    if _ML is None:
        import ml_dtypes

        _ML = ml_dtypes
    return _ML


def _bf16():
    return np.dtype(_mld().bfloat16)


def _f8():
    return np.dtype(_mld().float8_e4m3)


def _geom(nt):
    """Shared host/device geometry for nt 2048-col tiles per chunk row."""
    M = 2048 * nt              # points per chunk row j
    SB = 16 * nt               # superblocks of 1024 points
    PSEG = ((M // 8) * 9 // 8 + 511) // 512 * 512   # padded segment cols
    MS = 8 * PSEG              # sorted-padded cols per chunk row
    WR = MS // 512             # real 512-col reduce chunks
    NB = (WR * 8 + 127) // 128  # PSUM banks of [128,512] (16 chunks each)
    return M, SB, PSEG, MS, WR, NB


# ---------------------------------------------------------------------------
# walrus workaround: this toolchain allows only ONE sync-wait per
# instruction; spread extras onto preceding same-engine nops.
def _split_multi_waits(nc):
    from concourse import mybir

    n = 0
    for f in nc.m.functions:
        for bb in f.blocks:
            new_insts = []
            for ins in bb.instructions:
                si = getattr(ins, "sync_info", None)
                waits = list(si.on_wait) if si is not None and si.on_wait else []
                if len(waits) > 1:
                    for w in waits[:-1]:
                        n += 1
                        new_insts.append(
                            mybir.InstNoOp(
                                name=f"I-waitsplit-{n}",
                                engine=ins.engine,
                                bass_nofuse=True,
                                sync_info=mybir.SyncInfo(on_wait=[w], on_update=[]),
                            )
                        )
                    si.on_wait = waits[-1:]
                new_insts.append(ins)
            bb.instructions[:] = new_insts


# ---------------------------------------------------------------------------
# NTFF profiling hook (axon): lets run_bass_kernel_spmd(trace=True) work in
# this container. Harmless if the .so lacks the symbols.
def install_ntff_hook():
    try:
        import antenv

        if "antenv.axon_hooks" in sys.modules:
            return
        so_path = "/opt/axon/libaxon_pjrt.so"
        lib = ctypes.CDLL(so_path)
        if not hasattr(lib, "axon_start_nrt_profile"):
            return
        lib.axon_start_nrt_profile.argtypes = [
            ctypes.POINTER(ctypes.c_int64),
            ctypes.c_size_t,
        ]
        lib.axon_start_nrt_profile.restype = ctypes.c_int64
        lib.axon_stop_nrt_profile.argtypes = [ctypes.c_char_p]
        lib.axon_stop_nrt_profile.restype = ctypes.c_int64

        @contextlib.contextmanager
        def _hook(output_dir, device_ids):
            import jax

            jax.devices()
            if device_ids:
                ids = (ctypes.c_int64 * len(device_ids))(*device_ids)
                rc = lib.axon_start_nrt_profile(ids, len(device_ids))
            else:
                rc = lib.axon_start_nrt_profile(None, 0)
            if rc != 0:
                raise RuntimeError(f"axon_start_nrt_profile rc={rc}")
            try:
                yield
            finally:
                n = lib.axon_stop_nrt_profile(str(output_dir).encode())
                print(f"ntff profile: {n} file(s) -> {output_dir}", file=sys.stderr)

        mod = types.ModuleType("antenv.axon_hooks")
        mod.get_axon_ntff_profile_hook = lambda: _hook
        mod.set_axon_ntff_profile_hook = lambda h: None
        sys.modules["antenv.axon_hooks"] = mod
        antenv.axon_hooks = mod
    except Exception:
        pass


# ---------------------------------------------------------------------------
def build_nc(nt=16, num_devices=NCORES):
    """nt = number of 2048-col tiles of the unsorted layout (16 = full)."""
    import concourse.bass as bass
    import concourse.tile as tile
    from concourse import mybir

    assert nt % NCH == 0
    M, SB, PSEG, MS, WR, NB = _geom(nt)
    SBC = SB // NCH            # superblocks per DMA chunk
    MSC = MS // NCH            # sorted cols per DMA chunk
    assert MSC % 512 == 0 and PSEG % 512 == 0

    fp32 = mybir.dt.float32
    bf16 = mybir.dt.bfloat16
    fp8 = mybir.dt.float8e4

    nc = bass.Bass(
        "TRN2", target_bir_lowering=False, debug=False, num_devices=num_devices
    )

    x_t = nc.dram_tensor("x_t", [128, SB * 128], fp8, kind="ExternalInput").ap()
    oh_sb = nc.dram_tensor("oh_sb", [128, SB * 64], fp8, kind="ExternalInput").ap()
    x_s = nc.dram_tensor("x_s", [128, MS], fp8, kind="ExternalInput").ap()
    # bf16 copy of the back half of every segment (DVE square path: the
    # bf16 dtype unlocks the 4x/2x DVE perf modes, fp8 runs at 1x)
    x_sd = nc.dram_tensor("x_sd", [128, MS // 2], bf16, kind="ExternalInput").ap()
    # fp32 consts blob: id128 | id8 | r16 | m_inv
    blob32 = nc.dram_tensor("blob32", [128, 321], fp32, kind="ExternalInput").ap()
    # bf16 consts blob: red_d (4x32 flat)
    blob16 = nc.dram_tensor("blob16", [128, 128], bf16, kind="ExternalInput").ap()
    out_s = nc.dram_tensor("out_s", [D, K], fp32, kind="ExternalOutput").ap()
    out_var = nc.dram_tensor("out_var", [128, NB], fp32, kind="ExternalOutput").ap()

    with tile.TileContext(nc) as tc, contextlib.ExitStack() as ctx:
        # ---------------- pools
        xt_pool = ctx.enter_context(tc.tile_pool(name="xt", bufs=NCH))
        ohsb_pool = ctx.enter_context(tc.tile_pool(name="ohsb", bufs=NCH))
        xs_pool = ctx.enter_context(tc.tile_pool(name="xs", bufs=NCH))
        xsd_pool = ctx.enter_context(tc.tile_pool(name="xsd", bufs=NCH))
        singles = ctx.enter_context(tc.tile_pool(name="singles", bufs=1))
        sq_pool = ctx.enter_context(tc.tile_pool(name="sq", bufs=2))
        sbf_pool = ctx.enter_context(tc.tile_pool(name="sbf", bufs=2))
        me_pool = ctx.enter_context(tc.tile_pool(name="me", bufs=2))
        ps_cl_pool = ctx.enter_context(
            tc.tile_pool(name="ps_cl", bufs=1, space="PSUM")
        )
        ps_e_pool = ctx.enter_context(tc.tile_pool(name="ps_e", bufs=3, space="PSUM"))

        # ---------------- input DMAs (sync/SP ring drains in issue order:
        # phase-B data first, then const blobs, then the sorted x stream)
        xt = []
        ohsb = []
        for c in range(NCH):
            xtt = xt_pool.tile([128, SBC * 128], fp8, tag="xt")
            nc.sync.dma_start(out=xtt[:], in_=x_t[:, SBC * 128 * c : SBC * 128 * (c + 1)])
            xt.append(xtt)
            oht = ohsb_pool.tile([128, SBC * 64], fp8, tag="ohsb")
            nc.sync.dma_start(out=oht[:], in_=oh_sb[:, SBC * 64 * c : SBC * 64 * (c + 1)])
            ohsb.append(oht)

        b32 = singles.tile([128, 321], fp32)
        nc.sync.dma_start(out=b32[:], in_=blob32)
        b16 = singles.tile([128, 128], bf16)
        nc.sync.dma_start(out=b16[:], in_=blob16)
        id128_sb = b32[:, 0:128]
        id8_sb = b32[0:K, 128:136]
        r16_sb = b32[0:D, 192:320]
        m_inv_sb = b32[0:K, 320:321]
        red_sb = b16[:, 0:128]          # red_d[:, v] = cols 32v..32v+32

        xs = []
        xsd = []
        for c in range(NCH):
            xst = xs_pool.tile([128, MSC], fp8, tag="xs")
            nc.sync.dma_start(out=xst[:], in_=x_s[:, MSC * c : MSC * (c + 1)])
            xs.append(xst)
            xsdt = xsd_pool.tile([128, MSC // 2], bf16, tag="xsd")
            nc.sync.dma_start(
                out=xsdt[:], in_=x_sd[:, MSC // 2 * c : MSC // 2 * (c + 1)]
            )
            xsd.append(xsdt)

        # ---------------- phase B: cluster sums
        ps_cl = ps_cl_pool.tile([128, 512], fp32, tag="cl")
        for c in range(NCH):
            for l in range(SBC):
                g = SBC * c + l
                nc.tensor.matmul(
                    ps_cl[:, 0:64],
                    xt[c][:, 128 * l : 128 * (l + 1)],
                    ohsb[c][:, 64 * l : 64 * (l + 1)],
                    start=(g == 0),
                    stop=(g == SB - 1),
                )

        # ---------------- centers -> cbias (all engine ops, no DMAs)
        cl_sb = singles.tile([128, 64], fp32)
        nc.scalar.copy(out=cl_sb[:], in_=ps_cl[:, 0:64])
        # S[dd, k] = sum of the 8 diag blocks (rows 16b.., cols 8b..)
        for b in range(8):
            nc.tensor.matmul(
                ps_cl[0:D, 128:136],
                id128_sb[:, 16 * b : 16 * b + D],
                cl_sb[:, 8 * b : 8 * b + K],
                start=(b == 0),
                stop=(b == 7),
                skip_group_check=True,
            )
        s_dk = singles.tile([D, K], fp32)
        nc.scalar.copy(out=s_dk[:], in_=ps_cl[0:D, 128:136])
        nc.sync.dma_start(out=out_s, in_=s_dk[:])
        # s_rep[k, 16j+dd] = S[k, dd] / m_k
        nc.tensor.matmul(
            ps_cl[0:K, 256:384],
            s_dk[:],
            r16_sb,
            start=True,
            stop=True,
            skip_group_check=True,
        )
        s_rep = singles.tile([K, 128], fp32)
        nc.scalar.activation(
            out=s_rep[:],
            in_=ps_cl[0:K, 256:384],
            func=mybir.ActivationFunctionType.Copy,
            scale=m_inv_sb,
        )
        # cbias[16j+dd, k] = -c[k, dd]
        nc.tensor.matmul(
            ps_cl[:, 384:392],
            s_rep[:],
            id8_sb,
            start=True,
            stop=True,
            skip_group_check=True,
        )
        cbias = singles.tile([128, K], fp32)
        nc.scalar.activation(
            out=cbias[:],
            in_=ps_cl[:, 384:392],
            func=mybir.ActivationFunctionType.Copy,
            scale=-1.0,
        )

        # ---------------- pass 2 (sorted segments)
        # sq = (x - c_own)^2 via one ACT Square per label segment, then
        # dense col-tiled reduce matmuls: e-slot of sorted point (j, i):
        # w = i//512, bank = w//16, row = 32*((w%16)//4) + 8*(w%4) + j,
        # col = 512*bank + i%512.
        v_sb = singles.tile([128, NB], fp32)
        wpc = MSC // 512  # 512-col reduce chunks per DMA chunk

        def do_bank(bank):
            ps_e = ps_e_pool.tile([128, 512], fp32)
            for v in range(4):
                for cg in range(4):
                    w = 16 * bank + 4 * cg + v
                    wr = min(w, WR - 1)  # dummy chunks reuse the last real one
                    cc, wi = divmod(wr, wpc)
                    nc.tensor.matmul(
                        ps_e[32 * cg : 32 * cg + 32, :],
                        red_sb[:, 32 * v : 32 * (v + 1)],
                        sqs[cc][:, 512 * wi : 512 * (wi + 1)],
                        start=(v == 0),
                        stop=(v == 3),
                        tile_position=(0, 32 * cg),
                        skip_group_check=True,
                    )
            sbf = sbf_pool.tile([128, 512], bf16, tag="sbf")
            nc.scalar.activation(
                out=sbf[:], in_=ps_e[:], func=mybir.ActivationFunctionType.Sqrt
            )
            m_e = me_pool.tile([128, 512], bf16, tag="m_e")
            nc.vector.tensor_scalar(
                out=m_e[:],
                in0=sbf[:],
                scalar1=-float(DELTA_VAR),
                scalar2=0.0,
                op0=mybir.AluOpType.add,
                op1=mybir.AluOpType.max,
            )
            msq = me_pool.tile([128, 512], bf16, tag="msq")
            nc.vector.tensor_tensor(
                out=msq[:], in0=m_e[:], in1=m_e[:], op=mybir.AluOpType.mult
            )
            scr = me_pool.tile([128, 512], bf16, tag="scr")
            nc.vector.tensor_scalar(
                out=scr[:],
                in0=msq[:],
                scalar1=1.0,
                scalar2=None,
                op0=mybir.AluOpType.mult,
                op1=mybir.AluOpType.add,
                accum_out=v_sb[:, bank : bank + 1],
            )

        sqs = []
        bank_next = [0]
        for c in range(NCH):
            sq = sq_pool.tile([128, MSC], bf16, tag="sq")
            sk0 = (MSC * c) // PSEG
            for k in range(sk0, (MSC * (c + 1) - 1) // PSEG + 1):
                lo = max(MSC * c, PSEG * k) - MSC * c
                hi = min(MSC * (c + 1), PSEG * (k + 1)) - MSC * c
                mid = lo + (hi - lo) // 2
                nc.scalar.activation(
                    out=sq[:, lo:mid],
                    in_=xs[c][:, lo:mid],
                    func=mybir.ActivationFunctionType.Square,
                    bias=cbias[:, k : k + 1],
                )
                if mid < hi:
                    xo = (k - sk0) * (PSEG // 2)
                    dvp = me_pool.tile([128, PSEG // 2], bf16, tag="dvp")
                    nc.vector.tensor_scalar(
                        out=dvp[:, 0 : hi - mid],
                        in0=xsd[c][:, xo : xo + hi - mid],
                        scalar1=cbias[:, k : k + 1],
                        scalar2=None,
                        op0=mybir.AluOpType.add,
                    )
                    nc.vector.tensor_tensor(
                        out=sq[:, mid:hi],
                        in0=dvp[:, 0 : hi - mid],
                        in1=dvp[:, 0 : hi - mid],
                        op=mybir.AluOpType.mult,
                    )
            sqs.append(sq)
            # issue each bank's reduce/sqrt/V as soon as the squares it
            # needs are issued -- keeps sqrts out of the back of the ACT
            # FIFO so per-bank work overlaps the remaining squares
            while bank_next[0] < NB and min(16 * bank_next[0] + 15, WR - 1) // wpc <= c:
                do_bank(bank_next[0])
                bank_next[0] += 1

        assert bank_next[0] == NB
        nc.sync.dma_start(out=out_var, in_=v_sb[:])

    _split_multi_waits(nc)
    return nc


# ---------------------------------------------------------------------------
# host-side input prep
def prep_core_inputs(x_c, labels_c, nt=16):
    """x_c fp32 [16, NPTS] (d-major), labels_c int [NPTS] -> in_map."""
    M, SB, PSEG, MS, WR, NB = _geom(nt)
    NPTS = J * M
    bf = _bf16()
    f8 = _f8()
    x = np.ascontiguousarray(x_c, dtype=np.float32)
    lab = labels_c.astype(np.int64)
    assert x.shape == (D, NPTS) and lab.shape == (NPTS,)

    # x_t[nn, 128s+16b+dd] = x[dd, 1024s+128b+nn]
    x_t = np.ascontiguousarray(
        x.reshape(D, SB, 8, 128).transpose(3, 1, 2, 0).reshape(128, SB * 128)
    ).astype(f8)
    # oh_sb[nn, 64s+8b+k] = (lab[1024s+128b+nn] == k)
    l_sb = lab.reshape(SB, 8, 128)
    oh_sb = np.ascontiguousarray(
        (l_sb[:, :, :, None] == np.arange(K)).transpose(2, 0, 1, 3).reshape(128, SB * K * 8)
    ).astype(f8)

    # sorted-padded layout: per chunk j, points sorted by label, segment k
    # at cols [PSEG*k, PSEG*k + count[j,k]), pad cols = 0
    x_s = np.zeros((128, MS), dtype=np.float32)
    lab_s = np.full((J, MS), -1.0, dtype=np.float32)
    xr = x.reshape(D, J, M)
    for j in range(J):
        lj = lab[j * M : (j + 1) * M]
        order = np.argsort(lj, kind="stable")
        cnt = np.bincount(lj, minlength=K)
        assert cnt.max() <= PSEG, f"segment overflow {cnt.max()} > {PSEG}"
        xs_j = xr[:, j, order]          # [D, M] sorted by label
        ls_j = lj[order]
        pos = 0
        for k in range(K):
            seg = slice(PSEG * k, PSEG * k + cnt[k])
            x_s[16 * j : 16 * j + D, seg] = xs_j[:, pos : pos + cnt[k]]
            lab_s[j, seg] = ls_j[pos : pos + cnt[k]]
            pos += cnt[k]
    x_sf = x_s.reshape(128, K, PSEG)
    x_sd = np.ascontiguousarray(x_sf[:, :, PSEG // 2 :]).reshape(128, MS // 2).astype(bf)
    x_s = x_s.astype(f8)

    # red_d[16j+dd, 32v + 8v+j] = 1
    red_d = np.zeros((128, 128), dtype=np.float32)
    for j in range(J):
        for v in range(4):
            red_d[16 * j : 16 * j + D, 32 * v + 8 * v + j] = 1.0
    m = np.bincount(lab, minlength=K).astype(np.float64)
    m_inv = (1.0 / np.maximum(m, 1.0)).astype(np.float32).reshape(K, 1)

    blob32 = np.zeros((128, 321), dtype=np.float32)
    blob32[:, 0:128] = np.eye(128, dtype=np.float32)
    blob32[0:K, 128:136] = np.eye(K)
    blob32[0:D, 192:320] = np.tile(np.eye(D), (1, 8))   # r16[dd, 16j+dd] = 1
    blob32[0:K, 320:321] = m_inv
    blob16 = red_d.astype(bf)

    return {
        "x_t": x_t,
        "oh_sb": oh_sb,
        "x_s": x_s,
        "x_sd": x_sd,
        "blob32": blob32,
        "blob16": blob16,
    }


def vrow_label_map(nt=16):
    """label of e-row p in bank b (or -1 for dummy rows): every 512-col
    e-row holds points of one label since PSEG is 512-aligned."""
    M, SB, PSEG, MS, WR, NB = _geom(nt)
    lm = np.full((NB, 128), -1, dtype=np.int64)
    for w in range(WR):
        bank, r = divmod(w, 16)
        cg, v = divmod(r, 4)
        k = (512 * w) // PSEG % K
        lm[bank, 32 * cg + 8 * v : 32 * cg + 8 * v + J] = k
    return lm


def finish_host(s_list, var_list, counts_list, nt=16):
    """Combine per-core S [D, K] sums, V [128, NB] row-sums, counts [K]."""
    lm = vrow_label_map(nt)                    # [NB, 128]
    losses = []
    for S_dk, vparts, m in zip(s_list, var_list, counts_list):
        S = S_dk.astype(np.float64).T          # [K, D]
        m = m.astype(np.float64)
        centers = S / m[:, None]
        vp = vparts.astype(np.float64).T       # [NB, 128]
        V = np.array([vp[lm == k].sum() for k in range(K)])
        var_term = np.mean(V / m)
        dif = centers[None, :, :] - centers[:, None, :]
        dmat = np.sqrt((dif**2).sum(-1)) + np.eye(K) * DELTA_DIST
        dist_cost = np.clip(DELTA_DIST - dmat, 0.0, None) ** 2
        dist_term = dist_cost.sum() / (K * (K - 1))
        cn = np.sqrt((centers**2).sum(-1))
        reg_term = np.mean(np.clip(cn - np.sqrt(float(D)), 0.0, None) ** 2)
        losses.append(var_term + dist_term + reg_term)
    return np.float32(np.mean(losses))


# ---------------------------------------------------------------------------
_CACHE = {}


def _get_nc():
    if "nc" not in _CACHE:
        _CACHE["nc"] = build_nc(nt=16, num_devices=NCORES)
    return _CACHE["nc"]


def run_device(in_maps, trace=False):
    from concourse.bass_utils import run_bass_kernel_spmd

    if trace:
        install_ntff_hook()
    nc = _get_nc()
    return run_bass_kernel_spmd(
        nc, in_maps, core_ids=list(range(NCORES)), trace=trace
    )


def kernel(data, labels, n_clusters):
    assert int(n_clusters) == K
    assert data.shape == (B, D, HH, WW)
    x = np.asarray(data, dtype=np.float32).reshape(B, D, N)
    lab = np.asarray(labels).reshape(B, N)
    in_maps = [prep_core_inputs(x[c], lab[c]) for c in range(NCORES)]
    counts = [np.bincount(lab[c], minlength=K) for c in range(NCORES)]
    res = run_device(in_maps, trace=False)
    return finish_host(
        [r["out_s"] for r in res.results],
        [r["out_var"] for r in res.results],
        counts,
    )
